# revision 3
# baseline (speedup 1.0000x reference)
"""Distributed single-head attention for Trainium2 (8 NeuronCores).

Problem: B=4, S=2048, D=1024 fp32 attention:
    q = x@Wq+bq; k = x@Wk+bk; v = x@Wv+bv
    out = softmax(q k^T / sqrt(D) + mask) v

Sharding: data-parallel over (batch, query-half): core c handles batch
c//2, query rows [1024*(c%2), 1024*(c%2)+1024). Keys/values for the whole
sequence are computed on-core (K/V projection duplicated per core pair) so
no collectives are needed.

Per-core layouts (host-prepared):
  xt  bf16 [1024(d), 2048(s)]: x[b] rotated so the 1024 query rows come
      first, then transposed. Key order is a rotation of the original —
      softmax+PV are invariant to key permutation.
  wq/wk/wv bf16 [1024(d), 1024(e)]: natural lhsT for out[e,s] matmuls.
  bq2/bk2  f32 [128, 8]: bias chunk e  at [:, e] (per-partition bias).
  bvr bf16 [1, 1024]: V bias as a row (added via rank-1 matmul).
  maskp f32 [1024, 2048] (only when mask is nonzero): additive mask for
      this core's q rows, key-rotated, pre-divided by SCALE so the fused
      exp(SCALE*x) picks it up exactly.

On-chip per core:
  qT[e,q]  = Wq^T xT   (+bq)   -> bf16 SBUF
  kT[e,s]  = Wk^T xT   (+bk)   -> bf16 SBUF
  V[s,e]   = xT^T Wv   (+bv via ones-row matmul) -> bf16 SBUF
  per q-chunk (128 rows):
    scores[q,s] = qT^T kT  (fp32 PSUM, two 1024-wide halves)
    e = exp(SCALE*scores (+mask)), row-sums via ScalarE accum_out
    attnT = DMA-transpose(e) (bf16, 16x 128x128 tiles)
    o[q,e] = attnT^T V (fp32 PSUM), evicted with *1/rowsum fused
"""

from contextlib import ExitStack

import numpy as np
import ml_dtypes

import concourse.bass as bass
import concourse.tile as tile
import concourse.mybir as mybir
from concourse import bacc
from concourse.bass_utils import run_bass_kernel_spmd

BF16 = mybir.dt.bfloat16
F32 = mybir.dt.float32
AF = mybir.ActivationFunctionType

D = 1024  # model dim (= contraction dim for projections)
S = 2048  # full sequence (keys)
Q = 1024  # queries per core
P = 128  # partitions
ND = D // P  # 8 d-chunks
NS = S // P  # 16 key chunks
NQ = Q // P  # 8 query chunks
SCALE = 1.0 / float(np.sqrt(np.float32(D)))

_NC_CACHE: dict[bool, bacc.Bacc] = {}


def _build(use_mask: bool) -> bacc.Bacc:
    nc = bacc.Bacc("TRN2", target_bir_lowering=False, debug=False, num_devices=8)

    xt_d = nc.dram_tensor("xt", [D, S], BF16, kind="ExternalInput")
    wq_d = nc.dram_tensor("wq", [D, D], BF16, kind="ExternalInput")
    wk_d = nc.dram_tensor("wk", [D, D], BF16, kind="ExternalInput")
    wv_d = nc.dram_tensor("wv", [D, D], BF16, kind="ExternalInput")
    bq_d = nc.dram_tensor("bq2", [P, ND], F32, kind="ExternalInput")
    bk_d = nc.dram_tensor("bk2", [P, ND], F32, kind="ExternalInput")
    bv_d = nc.dram_tensor("bvr", [1, D], BF16, kind="ExternalInput")
    if use_mask:
        mask_d = nc.dram_tensor("maskp", [Q, S], F32, kind="ExternalInput")
    out_d = nc.dram_tensor("out", [Q, D], F32, kind="ExternalOutput")

    nbuf = 1 if use_mask else 2

    with tile.TileContext(nc) as tc, ExitStack() as ctx:
        xt_pool = ctx.enter_context(tc.tile_pool(name="xt", bufs=ND))
        wq_pool = ctx.enter_context(tc.tile_pool(name="wq", bufs=ND))
        wk_pool = ctx.enter_context(tc.tile_pool(name="wk", bufs=ND))
        wv_pool = ctx.enter_context(tc.tile_pool(name="wv", bufs=ND))
        qt_pool = ctx.enter_context(tc.tile_pool(name="qt", bufs=NQ))
        kt_pool = ctx.enter_context(tc.tile_pool(name="kt", bufs=ND))
        vt_pool = ctx.enter_context(tc.tile_pool(name="vt", bufs=NS))
        const_pool = ctx.enter_context(tc.tile_pool(name="const", bufs=1))
        exp_pool = ctx.enter_context(tc.tile_pool(name="exp", bufs=nbuf))
        at_pool = ctx.enter_context(tc.tile_pool(name="at", bufs=nbuf))
        stat_pool = ctx.enter_context(tc.tile_pool(name="stat", bufs=4 * nbuf))
        o_pool = ctx.enter_context(tc.tile_pool(name="o", bufs=2))
        if use_mask:
            m_pool = ctx.enter_context(tc.tile_pool(name="m", bufs=2))
        mm_psum = ctx.enter_context(tc.tile_pool(name="mmp", bufs=2, space="PSUM"))
        sc_psum = ctx.enter_context(tc.tile_pool(name="scp", bufs=2, space="PSUM"))

        # ---- loads ----
        xt = [xt_pool.tile([P, S], BF16, tag="xt", name=f"xt{i}") for i in range(ND)]
        for d in range(ND):
            nc.sync.dma_start(xt[d][:], xt_d[d * P : (d + 1) * P, :])
        wq = [wq_pool.tile([P, D], BF16, tag="wq", name=f"wq{i}") for i in range(ND)]
        wk = [wk_pool.tile([P, D], BF16, tag="wk", name=f"wk{i}") for i in range(ND)]
        wv = [wv_pool.tile([P, D], BF16, tag="wv", name=f"wv{i}") for i in range(ND)]
        for d in range(ND):
            nc.sync.dma_start(wq[d][:], wq_d[d * P : (d + 1) * P, :])
        for d in range(ND):
            nc.sync.dma_start(wk[d][:], wk_d[d * P : (d + 1) * P, :])
        for d in range(ND):
            nc.sync.dma_start(wv[d][:], wv_d[d * P : (d + 1) * P, :])
        bq_sb = const_pool.tile([P, ND], F32, tag="bq")
        bk_sb = const_pool.tile([P, ND], F32, tag="bk")
        bv_sb = const_pool.tile([1, D], BF16, tag="bv")
        ones_sb = const_pool.tile([1, P], BF16, tag="ones")
        nc.sync.dma_start(bq_sb[:], bq_d[:, :])
        nc.sync.dma_start(bk_sb[:], bk_d[:, :])
        nc.sync.dma_start(bv_sb[:], bv_d[:, :])
        nc.gpsimd.memset(ones_sb[:], 1.0)

        # ---- qT[e,q] = sum_d Wq[d,e]^T xT[d,q] + bq[e] ----
        qt = []
        for e in range(ND):
            ps = mm_psum.tile([P, Q], F32, tag="mmp")
            for d in range(ND):
                for n in range(2):
                    nc.tensor.matmul(
                        ps[:, n * 512 : (n + 1) * 512],
                        lhsT=wq[d][:, e * P : (e + 1) * P],
                        rhs=xt[d][:, n * 512 : (n + 1) * 512],
                        start=(d == 0),
                        stop=(d == ND - 1),
                    )
            t = qt_pool.tile([P, Q], BF16, tag="qt")
            nc.scalar.activation(t[:], ps[:], AF.Identity, bias=bq_sb[:, e : e + 1])
            qt.append(t)

        # ---- kT[e,s] = sum_d Wk[d,e]^T xT[d,s] + bk[e] ----
        kt = []
        for e in range(ND):
            t = kt_pool.tile([P, S], BF16, tag="kt")
            for half in range(2):
                ps = mm_psum.tile([P, Q], F32, tag="mmp")
                for d in range(ND):
                    for n in range(2):
                        nc.tensor.matmul(
                            ps[:, n * 512 : (n + 1) * 512],
                            lhsT=wk[d][:, e * P : (e + 1) * P],
                            rhs=xt[d][:, half * 1024 + n * 512 : half * 1024 + (n + 1) * 512],
                            start=(d == 0),
                            stop=(d == ND - 1),
                        )
                nc.scalar.activation(
                    t[:, half * 1024 : (half + 1) * 1024],
                    ps[:],
                    AF.Identity,
                    bias=bk_sb[:, e : e + 1],
                )
            kt.append(t)

        # ---- V[s,e] = sum_d xT[d,s]^T Wv[d,e] + bv[e] (ones-row matmul) ----
        vt = []
        for k in range(NS):
            ps = mm_psum.tile([P, D], F32, tag="mmp")
            for d in range(ND):
                for n in range(2):
                    nc.tensor.matmul(
                        ps[:, n * 512 : (n + 1) * 512],
                        lhsT=xt[d][:, k * P : (k + 1) * P],
                        rhs=wv[d][:, n * 512 : (n + 1) * 512],
                        start=(d == 0),
                        stop=False,
                    )
            for n in range(2):
                nc.tensor.matmul(
                    ps[:, n * 512 : (n + 1) * 512],
                    lhsT=ones_sb[:, 0:P],
                    rhs=bv_sb[:, n * 512 : (n + 1) * 512],
                    start=False,
                    stop=True,
                )
            t = vt_pool.tile([P, D], BF16, tag="vt")
            nc.vector.tensor_copy(t[:], ps[:])
            vt.append(t)

        # ---- attention, software-pipelined over 8 q-chunks ----
        def scores_phase(qc):
            """scores matmuls + exp(+mask) + row sums for q-chunk qc."""
            exp_sb = exp_pool.tile([P, S], BF16, tag="exp")
            sums = stat_pool.tile([P, 2], F32, tag="sums")
            for half in range(2):
                ps = sc_psum.tile([P, Q], F32, tag="scp")
                for e in range(ND):
                    for n in range(2):
                        nc.tensor.matmul(
                            ps[:, n * 512 : (n + 1) * 512],
                            lhsT=qt[e][:, qc * P : (qc + 1) * P],
                            rhs=kt[e][:, half * 1024 + n * 512 : half * 1024 + (n + 1) * 512],
                            start=(e == 0),
                            stop=(e == ND - 1),
                        )
                if use_mask:
                    mt = m_pool.tile([P, Q], F32, tag="m")
                    nc.sync.dma_start(
                        mt[:], mask_d[qc * P : (qc + 1) * P, half * 1024 : (half + 1) * 1024]
                    )
                    nc.vector.tensor_add(ps[:], ps[:], mt[:])
                nc.scalar.activation(
                    exp_sb[:, half * 1024 : (half + 1) * 1024],
                    ps[:],
                    AF.Exp,
                    scale=SCALE,
                    accum_out=sums[:, half : half + 1],
                )
            return exp_sb, sums

        def pv_phase(qc, exp_sb, sums):
            """transpose + PV + normalized eviction for q-chunk qc."""
            rsum = stat_pool.tile([P, 1], F32, tag="rsum")
            nc.vector.tensor_add(rsum[:], sums[:, 0:1], sums[:, 1:2])
            rinv = stat_pool.tile([P, 1], F32, tag="rinv")
            nc.vector.reciprocal(rinv[:], rsum[:])
            at_sb = at_pool.tile([P, S], BF16, tag="at")
            for k in range(NS):
                nc.sync.dma_start(
                    out=at_sb[:, k * P : (k + 1) * P],
                    in_=exp_sb[:, k * P : (k + 1) * P],
                    transpose=True,
                )
            pv = mm_psum.tile([P, D], F32, tag="mmp")
            for k in range(NS):
                for n in range(2):
                    nc.tensor.matmul(
                        pv[:, n * 512 : (n + 1) * 512],
                        lhsT=at_sb[:, k * P : (k + 1) * P],
                        rhs=vt[k][:, n * 512 : (n + 1) * 512],
                        start=(k == 0),
                        stop=(k == NS - 1),
                    )
            ot = o_pool.tile([P, D], F32, tag="o")
            nc.vector.tensor_scalar_mul(ot[:], pv[:], rinv[:])
            nc.sync.dma_start(out_d[qc * P : (qc + 1) * P, :], ot[:])

        # emit scores(qc+1) before pv(qc) so the PE never stalls waiting on
        # the exp/transpose of the current chunk
        pend = scores_phase(0)
        for qc in range(NQ):
            nxt = scores_phase(qc + 1) if qc + 1 < NQ else None
            pv_phase(qc, *pend)
            pend = nxt

    nc.compile()
    return nc


def _get_nc(use_mask: bool) -> bacc.Bacc:
    if use_mask not in _NC_CACHE:
        _NC_CACHE[use_mask] = _build(use_mask)
    return _NC_CACHE[use_mask]


def kernel(x, mask, Wq, bq, Wk, bk, Wv, bv):
    x = np.asarray(x, dtype=np.float32)
    mask = np.asarray(mask, dtype=np.float32)
    Wq = np.asarray(Wq, dtype=np.float32)
    bq = np.asarray(bq, dtype=np.float32)
    Wk = np.asarray(Wk, dtype=np.float32)
    bk = np.asarray(bk, dtype=np.float32)
    Wv = np.asarray(Wv, dtype=np.float32)
    bv = np.asarray(bv, dtype=np.float32)

    B = x.shape[0]
    use_mask = bool(np.any(mask))
    nc = _get_nc(use_mask)

    bf = ml_dtypes.bfloat16
    wq_b = Wq.astype(bf)
    wk_b = Wk.astype(bf)
    wv_b = Wv.astype(bf)
    bq2 = np.ascontiguousarray(bq.reshape(ND, P).T)
    bk2 = np.ascontiguousarray(bk.reshape(ND, P).T)
    bvr = bv.reshape(1, D).astype(bf)

    in_maps = []
    for c in range(8):
        b, h = divmod(c, 2)
        off = h * Q
        xb = x[b]
        x_rot = np.concatenate([xb[off:], xb[:off]], axis=0)  # queries first
        xt = np.ascontiguousarray(x_rot.T).astype(bf)
        im = {
            "xt": xt,
            "wq": wq_b,
            "wk": wk_b,
            "wv": wv_b,
            "bq2": bq2,
            "bk2": bk2,
            "bvr": bvr,
        }
        if use_mask:
            mrows = mask[off : off + Q]
            m_rot = np.concatenate([mrows[:, off:], mrows[:, :off]], axis=1)
            im["maskp"] = np.ascontiguousarray(m_rot / np.float32(SCALE)).astype(
                np.float32
            )
        in_maps.append(im)

    res = run_bass_kernel_spmd(nc, in_maps, core_ids=list(range(8)))

    out = np.empty((B, S, D), dtype=np.float32)
    for c in range(8):
        b, h = divmod(c, 2)
        out[b, h * Q : (h + 1) * Q, :] = res.results[c]["out"]
    return out


# revision 5
# speedup vs baseline: 1.3129x; 1.3129x over previous
"""Distributed single-head attention for Trainium2 (8 NeuronCores).

Problem: B=4, S=2048, D=1024 fp32 attention:
    q = x@Wq+bq; k = x@Wk+bk; v = x@Wv+bv
    out = softmax(q k^T / sqrt(D) + mask) v

Sharding: data-parallel over (batch, query-half): core c handles batch
c//2, query rows [1024*(c%2), 1024*(c%2)+1024). Keys/values for the whole
sequence are computed on-core (K/V projection duplicated per core pair) so
no collectives are needed.

Per-core layouts (host-prepared):
  xt  bf16 [1024(d), 2048(s)]: x[b] rotated so the 1024 query rows come
      first, then transposed. Key order is a rotation of the original —
      softmax+PV are invariant to key permutation.
  wq/wk/wv bf16 [1024(d), 1024(e)]: natural lhsT for out[e,s] matmuls.
  bq2/bk2  f32 [128, 8]: bias chunk e  at [:, e] (per-partition bias).
  bvr bf16 [1, 1024]: V bias as a row (added via rank-1 matmul).
  maskp f32 [1024, 2048] (only when mask is nonzero): additive mask for
      this core's q rows, key-rotated, pre-divided by SCALE so the fused
      exp(SCALE*x) picks it up exactly.

On-chip per core:
  qT[e,q]  = Wq^T xT   (+bq)   -> bf16 SBUF
  kT[e,s]  = Wk^T xT   (+bk)   -> bf16 SBUF
  V[s,e]   = xT^T Wv   (+bv via ones-row matmul) -> bf16 SBUF
  per q-chunk (128 rows):
    scores[q,s] = qT^T kT  (fp32 PSUM, two 1024-wide halves)
    e = exp(SCALE*scores (+mask)), row-sums via ScalarE accum_out
    attnT = DMA-transpose(e) (bf16, 16x 128x128 tiles)
    o[q,e] = attnT^T V (fp32 PSUM), evicted with *1/rowsum fused
"""

from contextlib import ExitStack

import numpy as np
import ml_dtypes

import concourse.bass as bass
import concourse.tile as tile
import concourse.mybir as mybir
from concourse import bacc
from concourse.bass_utils import run_bass_kernel_spmd

BF16 = mybir.dt.bfloat16
F32 = mybir.dt.float32
AF = mybir.ActivationFunctionType

D = 1024  # model dim (= contraction dim for projections)
S = 2048  # full sequence (keys)
Q = 1024  # queries per core
P = 128  # partitions
ND = D // P  # 8 d-chunks
NS = S // P  # 16 key chunks
NQ = Q // P  # 8 query chunks
SCALE = 1.0 / float(np.sqrt(np.float32(D)))

_NC_CACHE: dict[bool, bacc.Bacc] = {}


def _build(use_mask: bool) -> bacc.Bacc:
    nc = bacc.Bacc("TRN2", target_bir_lowering=False, debug=False, num_devices=8)

    xt_d = nc.dram_tensor("xt", [D, S], BF16, kind="ExternalInput")
    wq_d = nc.dram_tensor("wq", [D, D], BF16, kind="ExternalInput")
    wk_d = nc.dram_tensor("wk", [D, D], BF16, kind="ExternalInput")
    wv_d = nc.dram_tensor("wv", [D, D], BF16, kind="ExternalInput")
    bq_d = nc.dram_tensor("bq2", [P, ND], F32, kind="ExternalInput")
    bk_d = nc.dram_tensor("bk2", [P, ND], F32, kind="ExternalInput")
    bv_d = nc.dram_tensor("bvr", [1, D], BF16, kind="ExternalInput")
    if use_mask:
        mask_d = nc.dram_tensor("maskp", [Q, S], F32, kind="ExternalInput")
    out_d = nc.dram_tensor("out", [Q, D], F32, kind="ExternalOutput")

    nbuf = 1 if use_mask else 2

    with tile.TileContext(nc) as tc, ExitStack() as ctx:
        xt_pool = ctx.enter_context(tc.tile_pool(name="xt", bufs=ND))
        wq_pool = ctx.enter_context(tc.tile_pool(name="wq", bufs=ND))
        wk_pool = ctx.enter_context(tc.tile_pool(name="wk", bufs=ND))
        wv_pool = ctx.enter_context(tc.tile_pool(name="wv", bufs=ND))
        qt_pool = ctx.enter_context(tc.tile_pool(name="qt", bufs=NQ))
        kt_pool = ctx.enter_context(tc.tile_pool(name="kt", bufs=ND))
        vt_pool = ctx.enter_context(tc.tile_pool(name="vt", bufs=NS))
        const_pool = ctx.enter_context(tc.tile_pool(name="const", bufs=1))
        exp_pool = ctx.enter_context(tc.tile_pool(name="exp", bufs=nbuf))
        at_pool = ctx.enter_context(tc.tile_pool(name="at", bufs=nbuf))
        stat_pool = ctx.enter_context(tc.tile_pool(name="stat", bufs=4 * nbuf))
        o_pool = ctx.enter_context(tc.tile_pool(name="o", bufs=2))
        if use_mask:
            m_pool = ctx.enter_context(tc.tile_pool(name="m", bufs=2))
        mm_psum = ctx.enter_context(tc.tile_pool(name="mmp", bufs=2, space="PSUM"))
        sc_psum = ctx.enter_context(tc.tile_pool(name="scp", bufs=2, space="PSUM"))

        # ---- loads (interleaved so the first qT matmuls can start early) ----
        xt = [xt_pool.tile([P, S], BF16, tag="xt", name=f"xt{i}") for i in range(ND)]
        wq = [wq_pool.tile([P, D], BF16, tag="wq", name=f"wq{i}") for i in range(ND)]
        wk = [wk_pool.tile([P, D], BF16, tag="wk", name=f"wk{i}") for i in range(ND)]
        wv = [wv_pool.tile([P, D], BF16, tag="wv", name=f"wv{i}") for i in range(ND)]
        for d in range(ND):
            nc.sync.dma_start(xt[d][:], xt_d[d * P : (d + 1) * P, :])
            nc.scalar.dma_start(wq[d][:], wq_d[d * P : (d + 1) * P, :])
        for d in range(ND):
            nc.scalar.dma_start(wk[d][:], wk_d[d * P : (d + 1) * P, :])
        for d in range(ND):
            nc.sync.dma_start(wv[d][:], wv_d[d * P : (d + 1) * P, :])
        bq_sb = const_pool.tile([P, ND], F32, tag="bq")
        bk_sb = const_pool.tile([P, ND], F32, tag="bk")
        bv_sb = const_pool.tile([1, D], BF16, tag="bv")
        ones_sb = const_pool.tile([1, P], BF16, tag="ones")
        nc.sync.dma_start(bq_sb[:], bq_d[:, :])
        nc.sync.dma_start(bk_sb[:], bk_d[:, :])
        nc.sync.dma_start(bv_sb[:], bv_d[:, :])
        nc.gpsimd.memset(ones_sb[:], 1.0)

        # ---- qT[e,q] = sum_d Wq[d,e]^T xT[d,q] + bq[e] ----
        qt = []
        for e in range(ND):
            ps = mm_psum.tile([P, Q], F32, tag="mmp")
            for d in range(ND):
                for n in range(2):
                    nc.tensor.matmul(
                        ps[:, n * 512 : (n + 1) * 512],
                        lhsT=wq[d][:, e * P : (e + 1) * P],
                        rhs=xt[d][:, n * 512 : (n + 1) * 512],
                        start=(d == 0),
                        stop=(d == ND - 1),
                    )
            t = qt_pool.tile([P, Q], BF16, tag="qt")
            nc.scalar.activation(t[:], ps[:], AF.Identity, bias=bq_sb[:, e : e + 1])
            qt.append(t)

        # ---- kT[e,s] = sum_d Wk[d,e]^T xT[d,s] + bk[e] ----
        kt = []
        for e in range(ND):
            t = kt_pool.tile([P, S], BF16, tag="kt")
            for half in range(2):
                ps = mm_psum.tile([P, Q], F32, tag="mmp")
                for d in range(ND):
                    for n in range(2):
                        nc.tensor.matmul(
                            ps[:, n * 512 : (n + 1) * 512],
                            lhsT=wk[d][:, e * P : (e + 1) * P],
                            rhs=xt[d][:, half * 1024 + n * 512 : half * 1024 + (n + 1) * 512],
                            start=(d == 0),
                            stop=(d == ND - 1),
                        )
                nc.scalar.activation(
                    t[:, half * 1024 : (half + 1) * 1024],
                    ps[:],
                    AF.Identity,
                    bias=bk_sb[:, e : e + 1],
                )
            kt.append(t)

        # ---- V[s,e] = sum_d xT[d,s]^T Wv[d,e] + bv[e] (ones-row matmul) ----
        vt = []
        for k in range(NS):
            ps = mm_psum.tile([P, D], F32, tag="mmp")
            for d in range(ND):
                for n in range(2):
                    nc.tensor.matmul(
                        ps[:, n * 512 : (n + 1) * 512],
                        lhsT=xt[d][:, k * P : (k + 1) * P],
                        rhs=wv[d][:, n * 512 : (n + 1) * 512],
                        start=(d == 0),
                        stop=False,
                    )
            for n in range(2):
                nc.tensor.matmul(
                    ps[:, n * 512 : (n + 1) * 512],
                    lhsT=ones_sb[:, 0:P],
                    rhs=bv_sb[:, n * 512 : (n + 1) * 512],
                    start=False,
                    stop=True,
                )
            t = vt_pool.tile([P, D], BF16, tag="vt")
            nc.vector.tensor_copy(t[:], ps[:])
            vt.append(t)

        # ---- attention, software-pipelined over 8 q-chunks ----
        def scores_phase(qc):
            """scores matmuls + exp(+mask) + row sums for q-chunk qc."""
            exp_sb = exp_pool.tile([P, S], BF16, tag="exp")
            sums = stat_pool.tile([P, 2], F32, tag="sums")
            for half in range(2):
                ps = sc_psum.tile([P, Q], F32, tag="scp")
                for e in range(ND):
                    for n in range(2):
                        nc.tensor.matmul(
                            ps[:, n * 512 : (n + 1) * 512],
                            lhsT=qt[e][:, qc * P : (qc + 1) * P],
                            rhs=kt[e][:, half * 1024 + n * 512 : half * 1024 + (n + 1) * 512],
                            start=(e == 0),
                            stop=(e == ND - 1),
                        )
                if use_mask:
                    mt = m_pool.tile([P, Q], F32, tag="m")
                    nc.sync.dma_start(
                        mt[:], mask_d[qc * P : (qc + 1) * P, half * 1024 : (half + 1) * 1024]
                    )
                    nc.vector.tensor_add(ps[:], ps[:], mt[:])
                nc.scalar.activation(
                    exp_sb[:, half * 1024 : (half + 1) * 1024],
                    ps[:],
                    AF.Exp,
                    scale=SCALE,
                    accum_out=sums[:, half : half + 1],
                )
            return exp_sb, sums

        def pv_phase(qc, exp_sb, sums):
            """transpose + PV + normalized eviction for q-chunk qc."""
            rsum = stat_pool.tile([P, 1], F32, tag="rsum")
            nc.vector.tensor_add(rsum[:], sums[:, 0:1], sums[:, 1:2])
            rinv = stat_pool.tile([P, 1], F32, tag="rinv")
            nc.vector.reciprocal(rinv[:], rsum[:])
            at_sb = at_pool.tile([P, S], BF16, tag="at")
            # one xbar transpose for all 16 chunks: out[p, c, q] = exp[q, c*128+p]
            nc.sync.dma_start(
                out=at_sb.rearrange("p (c q) -> p c q", q=P),
                in_=exp_sb[:, :],
                transpose=True,
            )
            pv = mm_psum.tile([P, D], F32, tag="mmp")
            for k in range(NS):
                for n in range(2):
                    nc.tensor.matmul(
                        pv[:, n * 512 : (n + 1) * 512],
                        lhsT=at_sb[:, k * P : (k + 1) * P],
                        rhs=vt[k][:, n * 512 : (n + 1) * 512],
                        start=(k == 0),
                        stop=(k == NS - 1),
                    )
            ot = o_pool.tile([P, D], F32, tag="o")
            nc.vector.tensor_scalar_mul(ot[:], pv[:], rinv[:])
            nc.sync.dma_start(out_d[qc * P : (qc + 1) * P, :], ot[:])

        # emit scores(qc+1) before pv(qc) so the PE never stalls waiting on
        # the exp/transpose of the current chunk
        pend = scores_phase(0)
        for qc in range(NQ):
            nxt = scores_phase(qc + 1) if qc + 1 < NQ else None
            pv_phase(qc, *pend)
            pend = nxt

    nc.compile()
    return nc


def _get_nc(use_mask: bool) -> bacc.Bacc:
    if use_mask not in _NC_CACHE:
        _NC_CACHE[use_mask] = _build(use_mask)
    return _NC_CACHE[use_mask]


def kernel(x, mask, Wq, bq, Wk, bk, Wv, bv):
    x = np.asarray(x, dtype=np.float32)
    mask = np.asarray(mask, dtype=np.float32)
    Wq = np.asarray(Wq, dtype=np.float32)
    bq = np.asarray(bq, dtype=np.float32)
    Wk = np.asarray(Wk, dtype=np.float32)
    bk = np.asarray(bk, dtype=np.float32)
    Wv = np.asarray(Wv, dtype=np.float32)
    bv = np.asarray(bv, dtype=np.float32)

    B = x.shape[0]
    use_mask = bool(np.any(mask))
    nc = _get_nc(use_mask)

    bf = ml_dtypes.bfloat16
    wq_b = Wq.astype(bf)
    wk_b = Wk.astype(bf)
    wv_b = Wv.astype(bf)
    bq2 = np.ascontiguousarray(bq.reshape(ND, P).T)
    bk2 = np.ascontiguousarray(bk.reshape(ND, P).T)
    bvr = bv.reshape(1, D).astype(bf)

    in_maps = []
    for c in range(8):
        b, h = divmod(c, 2)
        off = h * Q
        xb = x[b]
        x_rot = np.concatenate([xb[off:], xb[:off]], axis=0)  # queries first
        xt = np.ascontiguousarray(x_rot.T).astype(bf)
        im = {
            "xt": xt,
            "wq": wq_b,
            "wk": wk_b,
            "wv": wv_b,
            "bq2": bq2,
            "bk2": bk2,
            "bvr": bvr,
        }
        if use_mask:
            mrows = mask[off : off + Q]
            m_rot = np.concatenate([mrows[:, off:], mrows[:, :off]], axis=1)
            im["maskp"] = np.ascontiguousarray(m_rot / np.float32(SCALE)).astype(
                np.float32
            )
        in_maps.append(im)

    res = run_bass_kernel_spmd(nc, in_maps, core_ids=list(range(8)))

    out = np.empty((B, S, D), dtype=np.float32)
    for c in range(8):
        b, h = divmod(c, 2)
        out[b, h * Q : (h + 1) * Q, :] = res.results[c]["out"]
    return out
